# revision 32
# baseline (speedup 1.0000x reference)
"""Trainium2 Bass kernel for a dense transformer block (nn_Block_29583734734992).

Reference computation (fp32):
    resid = resid + Attn(LN1(resid))          # 16 heads, d_head 64, causal
    resid = resid + MLP(LN2(resid)) + b_out   # d_mlp 4096, tanh-gelu

Sharding over 8 NeuronCores:
  - Attention phase: head-parallel. Core c computes heads (2c, 2c+1) for BOTH
    batch elements over the full 2048-token sequence. Scores/softmax/z are
    computed per head in transposed layout (d on partitions).
  - One 8-rank AllToAll reshards z from head-major to token-major: token shard
    j = (batch j//4, tokens 512*(j%4) .. +512). Core c ends up with all 16
    heads for its 512-token shard.
  - Post phase: token-parallel. Each core does the o-projection, residual add,
    LN2 and the full MLP for its 512 tokens, writing a [512, 1024] output
    shard that the host reassembles.

Numerics: all matmuls run as float32r (1 cycle/row for free dim >= 256);
LN scale/bias, the 1/sqrt(64) softmax scale and b_in are folded into weights /
activation biases on the host. Softmax skips max-subtraction (scores are tiny)
and applies the causal mask multiplicatively after exp; the per-query softmax
denominator comes from an extra ones-column appended to V's stationary operand.
"""

import sys

for _p in ("/opt/trn_rl_repo", "/root/.axon_site/_ro/trn_rl_repo"):
    if _p not in sys.path:
        sys.path.insert(0, _p)

import ml_dtypes
import numpy as np

import concourse.bass as bass
import concourse.mybir as mybir
import concourse.tile as tile
from concourse import bacc
from concourse.bass_utils import run_bass_kernel_spmd

F32 = mybir.dt.float32
F32R = mybir.dt.float32r
BF16 = mybir.dt.bfloat16
AF = mybir.ActivationFunctionType
OP = mybir.AluOpType

N_CORES = 8
B, S, D = 2, 2048, 1024
H, DH, DM = 16, 64, 4096
EPS = 1e-5
HPC = H // N_CORES  # heads per core = 2
TSH = (B * S) // N_CORES  # tokens per core post-A2A = 512
NT = S // 128  # 16 token tiles per batch
ND = D // 128  # 8 d_model chunks
NM = DM // 128  # 32 d_mlp chunks
NQC = S // 512  # 4 query chunks of 512
NKC = S // 128  # 16 key chunks of 128

# Set True to compute gelu explicitly as x + x*tanh-part (0.5 folded into
# w_out) instead of the ACT Gelu_apprx_tanh LUT.
EXPLICIT_GELU = False


def build_nc():
    nc = bacc.Bacc("TRN2", target_bir_lowering=False, debug=False, num_devices=N_CORES)

    x_all = nc.dram_tensor("x_all", [B, S, D], F32, kind="ExternalInput")
    resid_mine = nc.dram_tensor("resid_mine", [TSH, D], F32, kind="ExternalInput")
    wqkv = nc.dram_tensor("wqkv", [ND, 128, 3 * 2 * DH], F32R, kind="ExternalInput")
    bqkv = nc.dram_tensor("bqkv", [128, 3], F32, kind="ExternalInput")
    wo = nc.dram_tensor("wo", [ND, 128, D], F32R, kind="ExternalInput")
    win = nc.dram_tensor("win", [NM, ND, 128, 128], F32R, kind="ExternalInput")
    bin_ = nc.dram_tensor("bin", [NM, 128, 1], F32, kind="ExternalInput")
    wout = nc.dram_tensor("wout", [NM, 128, D], BF16, kind="ExternalInput")
    bout = nc.dram_tensor("bout", [1, D], F32R, kind="ExternalInput")
    masks = nc.dram_tensor("masks", [4, 128, 512], F32R, kind="ExternalInput")
    ident = nc.dram_tensor("ident", [128, 128], F32, kind="ExternalInput")
    ones = nc.dram_tensor("ones", [1, 128], F32R, kind="ExternalInput")
    vinit = nc.dram_tensor("vinit", [128, HPC, NKC, DH + 1], F32R, kind="ExternalInput")
    y = nc.dram_tensor("y", [TSH, D], F32, kind="ExternalOutput")

    with tile.TileContext(nc) as tc:
        with (
            tc.tile_pool(name="singles", bufs=1) as singles,
            tc.tile_pool(name="dram", bufs=1, space="DRAM") as dram,
        ):
            a2a_in = dram.tile([N_CORES, HPC * DH, 512], F32R)
            a2a_out = dram.tile([N_CORES, HPC * DH, 512], F32R)

            ident_sb = singles.tile([128, 128], F32)
            nc.sync.dma_start(ident_sb[:], ident[:])
            mask_sb = singles.tile([128, 4, 512], F32R)
            for p in range(4):
                nc.sync.dma_start(mask_sb[:, p, :], masks[p])
            wqkv_sb = singles.tile([128, ND, 3 * 2 * DH], F32R)
            nc.sync.dma_start(wqkv_sb[:], wqkv.rearrange("c p f -> p c f"))
            bqkv_sb = singles.tile([128, 3], F32)
            nc.sync.dma_start(bqkv_sb[:], bqkv[:])
            eps_sb = singles.tile([128, 1], F32)
            nc.vector.memset(eps_sb[:], EPS)
            bout_sb = singles.tile([1, D], F32R)
            nc.sync.dma_start(bout_sb[:], bout[:])
            ones_sb = singles.tile([1, 128], F32R)
            nc.sync.dma_start(ones_sb[:], ones[:])

            # ---------------- attention phase (head-parallel) ----------------
            with (
                tc.tile_pool(name="attn_x", bufs=2) as axp,
                tc.tile_pool(name="attn_big", bufs=1) as abig,
                tc.tile_pool(name="attn_qkv", bufs=2) as aqkv,
                tc.tile_pool(name="attn_vT", bufs=1) as avt,
                tc.tile_pool(name="attn_sm", bufs=3) as asm,
                tc.tile_pool(name="attn_ps", bufs=2, space="PSUM") as aps,
            ):
                for b in range(B):
                    xlnT = abig.tile([128, ND, S], F32R, tag="xlnT")
                    # LN1 over all tokens of batch b, then transpose into xlnT
                    for tci in range(NT):
                        xt = axp.tile([128, D], F32, tag="xt")
                        nc.sync.dma_start(xt[:], x_all[b, tci * 128 : (tci + 1) * 128, :])
                        stats = axp.tile([128, 2, 6], F32, tag="stats")
                        nc.vector.bn_stats(stats[:, 0, :], xt[:, 0:512])
                        nc.vector.bn_stats(stats[:, 1, :], xt[:, 512:1024])
                        mv = axp.tile([128, 2], F32, tag="mv")
                        nc.vector.bn_aggr(mv[:], stats[:])
                        std = axp.tile([128, 1], F32, tag="std")
                        nc.scalar.activation(std[:], mv[:, 1:2], AF.Sqrt, bias=eps_sb[:])
                        rstd = axp.tile([128, 1], F32, tag="rstd")
                        nc.vector.reciprocal(rstd[:], std[:])
                        xln = axp.tile([128, D], F32, tag="xln")
                        nc.vector.tensor_scalar(
                            out=xln[:],
                            in0=xt[:],
                            scalar1=mv[:, 0:1],
                            scalar2=rstd[:],
                            op0=OP.subtract,
                            op1=OP.mult,
                        )
                        for dc in range(ND):
                            tp = aps.tile([128, 128], F32, tag="smallps")
                            nc.tensor.transpose(
                                tp[:], xln[:, dc * 128 : (dc + 1) * 128], ident_sb[:]
                            )
                            nc.vector.tensor_copy(
                                xlnT[:, dc, tci * 128 : (tci + 1) * 128], tp[:]
                            )

                    # QKV^T for this core's 2 heads: [128 (2h x 64), S] each
                    qT = aqkv.tile([128, S], F32R, tag="qT")
                    kT = aqkv.tile([128, S], F32R, tag="kT")
                    vT = avt.tile([128, S], F32, tag="vT")
                    dests = [qT, kT, vT]
                    for p in range(3):
                        for tcol in range(NQC):
                            ps = aps.tile([128, 512], F32, tag="qkvps")
                            for dc in range(ND):
                                nc.tensor.matmul(
                                    ps[:],
                                    wqkv_sb[:, dc, p * 128 : (p + 1) * 128],
                                    xlnT[:, dc, tcol * 512 : (tcol + 1) * 512],
                                    start=(dc == 0),
                                    stop=(dc == ND - 1),
                                )
                            nc.vector.tensor_scalar_add(
                                out=dests[p][:, tcol * 512 : (tcol + 1) * 512],
                                in0=ps[:],
                                scalar1=bqkv_sb[:, p : p + 1],
                            )

                    # V token-major with an appended ones column: [128k, kc, 65]
                    vt = aqkv.tile([128, HPC, NKC, DH + 1], F32R, tag="vt")
                    nc.sync.dma_start(vt[:], vinit[:])
                    for h in range(HPC):
                        for kc in range(NKC):
                            tp = aps.tile([128, 128], F32, tag="smallps")
                            nc.tensor.transpose(
                                tp[:, 0:DH],
                                vT[h * DH : (h + 1) * DH, kc * 128 : (kc + 1) * 128],
                                ident_sb[h * DH : (h + 1) * DH, h * DH : (h + 1) * DH],
                            )
                            nc.vector.tensor_copy(vt[:, h, kc, 0:DH], tp[:, 0:DH])

                    # causal attention, z^T per head, normalize, stage for A2A
                    for h in range(HPC):
                        hs = slice(h * DH, (h + 1) * DH)
                        for qc in range(NQC):
                            zp = aps.tile([DH + 1, 512], F32, tag="zpsum")
                            nkc = 4 * qc + 4
                            for kc in range(nkc):
                                sp = aps.tile([128, 512], F32, tag="spsum")
                                nc.tensor.matmul(
                                    sp[:],
                                    kT[hs, kc * 128 : (kc + 1) * 128],
                                    qT[hs, qc * 512 : (qc + 1) * 512],
                                    start=True,
                                    stop=True,
                                )
                                es = asm.tile([128, 512], F32R, tag="es")
                                nc.scalar.activation(es[:], sp[:], AF.Exp)
                                if kc >= 4 * qc:
                                    nc.gpsimd.tensor_tensor(
                                        es[:], es[:], mask_sb[:, kc - 4 * qc, :], OP.mult
                                    )
                                nc.tensor.matmul(
                                    zp[:],
                                    vt[:, h, kc, :],
                                    es[:],
                                    start=(kc == 0),
                                    stop=(kc == nkc - 1),
                                )
                            recip = asm.tile([1, 512], F32R, tag="recip")
                            with nc.allow_low_precision(
                                reason="fp32r softmax denom; rounding loss is ~1e-7 rel"
                            ):
                                nc.vector.reciprocal(recip[:], zp[DH : DH + 1, :])
                            bc = aps.tile([DH, 512], F32, tag="smallps")
                            nc.tensor.matmul(
                                bc[:],
                                ones_sb[:, 0:DH],
                                recip[:],
                                start=True,
                                stop=True,
                            )
                            zn = asm.tile([DH, 512], F32R, tag="zn")
                            with nc.allow_low_precision(
                                reason="fp32r z output; rounding loss is ~1e-7 rel"
                            ):
                                nc.vector.tensor_copy(zn[:], zp[0:DH, :])
                                nc.vector.tensor_tensor(zn[:], zn[:], bc[:], OP.mult)
                            nc.sync.dma_start(
                                a2a_in[b * 4 + qc, h * DH : (h + 1) * DH, :], zn[:]
                            )

            nc.gpsimd.collective_compute(
                "AllToAll",
                OP.bypass,
                replica_groups=[list(range(N_CORES))],
                ins=[a2a_in[:]],
                outs=[a2a_out[:]],
            )

            # ---------------- post phase (token-parallel) ----------------
            with (
                tc.tile_pool(name="post_w", bufs=3) as pw,
                tc.tile_pool(name="post_big", bufs=1) as pbig,
                tc.tile_pool(name="post_t", bufs=3) as pt,
                tc.tile_pool(name="post_ps", bufs=2, space="PSUM") as pps,
            ):
                resid2 = pbig.tile([128, 4, D], F32, tag="resid2")

                # broadcast b_out across partitions once: [1, D] -> [128, D]
                bout_full = pbig.tile([128, D], F32, tag="bout_full")
                for dc2 in range(2):
                    bps = pps.tile([128, 512], F32, tag="ps1")
                    nc.tensor.matmul(
                        bps[:],
                        ones_sb[:],
                        bout_sb[:, dc2 * 512 : (dc2 + 1) * 512],
                        start=True,
                        stop=True,
                    )
                    nc.vector.tensor_copy(bout_full[:, dc2 * 512 : (dc2 + 1) * 512], bps[:])

                # o-projection + residual add -> resid2
                with tc.tile_pool(name="post_o", bufs=1) as po:
                    zt = po.tile([128, N_CORES, 512], F32R, tag="zt")
                    for i in range(N_CORES):
                        nc.sync.dma_start(zt[:, i, :], a2a_out[i])
                    wo_sb = po.tile([128, ND, D], F32R, tag="wo")
                    nc.sync.dma_start(wo_sb[:], wo.rearrange("c p f -> p c f"))
                    for tsub in range(4):
                        for dc2 in range(2):
                            op_ = pps.tile([128, 512], F32, tag="ps1")
                            for hd in range(ND):
                                nc.tensor.matmul(
                                    op_[:],
                                    zt[:, hd, tsub * 128 : (tsub + 1) * 128],
                                    wo_sb[:, hd, dc2 * 512 : (dc2 + 1) * 512],
                                    start=(hd == 0),
                                    stop=(hd == ND - 1),
                                )
                            rs = pt.tile([128, 512], F32, tag="rs")
                            nc.sync.dma_start(
                                rs[:],
                                resid_mine[
                                    tsub * 128 : (tsub + 1) * 128,
                                    dc2 * 512 : (dc2 + 1) * 512,
                                ],
                            )
                            nc.vector.tensor_tensor(
                                resid2[:, tsub, dc2 * 512 : (dc2 + 1) * 512],
                                op_[:],
                                rs[:],
                                OP.add,
                            )

                # LN2 + transpose -> xln2T [128, ND, 512]
                xln2T = pbig.tile([128, ND, 512], F32R, tag="xln2T")
                for tsub in range(4):
                    stats = pt.tile([128, 2, 6], F32, tag="stats2")
                    nc.vector.bn_stats(stats[:, 0, :], resid2[:, tsub, 0:512])
                    nc.vector.bn_stats(stats[:, 1, :], resid2[:, tsub, 512:1024])
                    mv = pt.tile([128, 2], F32, tag="mv2")
                    nc.vector.bn_aggr(mv[:], stats[:])
                    std = pt.tile([128, 1], F32, tag="std2")
                    nc.scalar.activation(std[:], mv[:, 1:2], AF.Sqrt, bias=eps_sb[:])
                    rstd = pt.tile([128, 1], F32, tag="rstd2")
                    nc.vector.reciprocal(rstd[:], std[:])
                    xln2 = pt.tile([128, D], F32, tag="xln2")
                    nc.vector.tensor_scalar(
                        out=xln2[:],
                        in0=resid2[:, tsub, :],
                        scalar1=mv[:, 0:1],
                        scalar2=rstd[:],
                        op0=OP.subtract,
                        op1=OP.mult,
                    )
                    for dc in range(ND):
                        tp = pps.tile([128, 128], F32, tag="ps1")
                        nc.tensor.transpose(
                            tp[:], xln2[:, dc * 128 : (dc + 1) * 128], ident_sb[:]
                        )
                        nc.vector.tensor_copy(
                            xln2T[:, dc, tsub * 128 : (tsub + 1) * 128], tp[:]
                        )

                # MLP pass A: h1^T per m-chunk -> gelu -> gT; accumulate out d 0:512
                gT = pbig.tile([128, NM, 512], BF16, tag="gT")
                acc = pps.tile([128, 4, 512], F32, tag="acc", bufs=1)
                for m in range(NM):
                    wi = pw.tile([128, ND, 128], F32R, tag="wi")
                    nc.sync.dma_start(wi[:], win[m].rearrange("c p f -> p c f"))
                    bi = pw.tile([128, 1], F32, tag="bi")
                    nc.sync.dma_start(bi[:], bin_[m])
                    h1 = pps.tile([128, 512], F32, tag="ps1")
                    for dc in range(ND):
                        nc.tensor.matmul(
                            h1[:],
                            wi[:, dc, :],
                            xln2T[:, dc, :],
                            start=(dc == 0),
                            stop=(dc == ND - 1),
                        )
                    if EXPLICIT_GELU:
                        # g = x*(1 + tanh(c*(x + a*x^3))), with x = h1 + b_in
                        # (the 0.5 is folded into w_out on the host)
                        xg = pt.tile([128, 512], F32, tag="xg")
                        nc.vector.tensor_scalar_add(out=xg[:], in0=h1[:], scalar1=bi[:])
                        x2 = pt.tile([128, 512], F32, tag="x2")
                        nc.vector.tensor_tensor(x2[:], xg[:], xg[:], OP.mult)
                        nc.vector.tensor_scalar(
                            out=x2[:],
                            in0=x2[:],
                            scalar1=0.044715,
                            scalar2=1.0,
                            op0=OP.mult,
                            op1=OP.add,
                        )
                        nc.vector.tensor_tensor(x2[:], x2[:], xg[:], OP.mult)
                        th = pt.tile([128, 512], F32, tag="th")
                        nc.scalar.activation(
                            th[:], x2[:], AF.Tanh, scale=0.7978845608028654
                        )
                        nc.vector.tensor_scalar_add(out=th[:], in0=th[:], scalar1=1.0)
                        nc.vector.tensor_tensor(gT[:, m, :], th[:], xg[:], OP.mult)
                    else:
                        nc.scalar.activation(
                            gT[:, m, :], h1[:], AF.Gelu_apprx_tanh, bias=bi[:]
                        )
                    wu = pw.tile([128, 512], BF16, tag="wu")
                    nc.sync.dma_start(wu[:], wout[m, :, 0:512])
                    for tsub in range(4):
                        nc.tensor.matmul(
                            acc[:, tsub, :],
                            gT[:, m, tsub * 128 : (tsub + 1) * 128],
                            wu[:],
                            start=(m == 0),
                            stop=(m == NM - 1),
                        )
                for tsub in range(4):
                    ot = pt.tile([128, 512], F32, tag="ot")
                    nc.vector.tensor_tensor(
                        ot[:], acc[:, tsub, :], resid2[:, tsub, 0:512], OP.add
                    )
                    nc.vector.tensor_tensor(ot[:], ot[:], bout_full[:, 0:512], OP.add)
                    nc.sync.dma_start(
                        y[tsub * 128 : (tsub + 1) * 128, 0:512], ot[:]
                    )

                # MLP pass B: same gT, out d 512:1024
                acc2 = pps.tile([128, 4, 512], F32, tag="acc", bufs=1)
                for m in range(NM):
                    wu = pw.tile([128, 512], BF16, tag="wu")
                    nc.sync.dma_start(wu[:], wout[m, :, 512:1024])
                    for tsub in range(4):
                        nc.tensor.matmul(
                            acc2[:, tsub, :],
                            gT[:, m, tsub * 128 : (tsub + 1) * 128],
                            wu[:],
                            start=(m == 0),
                            stop=(m == NM - 1),
                        )
                for tsub in range(4):
                    ot = pt.tile([128, 512], F32, tag="ot")
                    nc.vector.tensor_tensor(
                        ot[:], acc2[:, tsub, :], resid2[:, tsub, 512:1024], OP.add
                    )
                    nc.vector.tensor_tensor(ot[:], ot[:], bout_full[:, 512:1024], OP.add)
                    nc.sync.dma_start(
                        y[tsub * 128 : (tsub + 1) * 128, 512:1024], ot[:]
                    )

    nc.compile()
    return nc


def _prep_inputs(inputs):
    """Host-side weight folding; returns per-core in_maps."""
    f32 = np.float32
    resid = np.asarray(inputs["resid"], f32)
    w_q = np.asarray(inputs["w_q"], f32)
    w_k = np.asarray(inputs["w_k"], f32)
    w_v = np.asarray(inputs["w_v"], f32)
    w_o = np.asarray(inputs["w_o"], f32)
    ln1_w = np.asarray(inputs["ln1_w"], f32)
    ln1_b = np.asarray(inputs["ln1_b"], f32)
    ln2_w = np.asarray(inputs["ln2_w"], f32)
    ln2_b = np.asarray(inputs["ln2_b"], f32)
    w_in = np.asarray(inputs["w_in"], f32)
    b_in = np.asarray(inputs["b_in"], f32)
    w_out = np.asarray(inputs["w_out"], f32)
    b_out = np.asarray(inputs["b_out"], f32)

    sm = 1.0 / np.sqrt(DH)
    win_f = ln2_w[:, None] * w_in  # [D, DM]
    bin_f = ln2_b @ w_in + b_in  # [DM]
    wout_f = (0.5 * w_out if EXPLICIT_GELU else w_out).astype(f32)  # [DM, D]

    win_host = np.ascontiguousarray(
        win_f.reshape(ND, 128, NM, 128).transpose(2, 0, 1, 3)
    )
    bin_host = np.ascontiguousarray(bin_f.reshape(NM, 128, 1))
    wout_host = np.ascontiguousarray(
        wout_f.reshape(NM, 128, D).astype(ml_dtypes.bfloat16)
    )
    wo_host = np.ascontiguousarray(
        w_o.reshape(H * DH, D).reshape(ND, 128, D)
    )
    bout_host = np.ascontiguousarray(b_out.reshape(1, D))

    ones_host = np.ones((1, 128), f32)
    vinit_host = np.zeros((128, HPC, NKC, DH + 1), f32)
    vinit_host[:, :, :, DH] = 1.0

    masks = np.zeros((4, 128, 512), f32)
    for p in range(4):
        kk = 128 * p + np.arange(128)[:, None]
        qq = np.arange(512)[None, :]
        masks[p] = (kk <= qq).astype(f32)
    ident = np.eye(128, dtype=f32)

    in_maps = []
    for c in range(N_CORES):
        h0 = HPC * c
        wq2 = np.concatenate([w_q[h0 + i] for i in range(HPC)], axis=1)  # [D, 128]
        wk2 = np.concatenate([w_k[h0 + i] for i in range(HPC)], axis=1)
        wv2 = np.concatenate([w_v[h0 + i] for i in range(HPC)], axis=1)
        wq_f = ln1_w[:, None] * wq2 * sm
        wk_f = ln1_w[:, None] * wk2
        wv_f = ln1_w[:, None] * wv2
        bq = (ln1_b @ wq2) * sm
        bk = ln1_b @ wk2
        bv = ln1_b @ wv2
        wqkv_host = np.ascontiguousarray(
            np.concatenate([wq_f, wk_f, wv_f], axis=1).reshape(ND, 128, 384)
        )
        bqkv_host = np.ascontiguousarray(np.stack([bq, bk, bv], axis=1))  # [128, 3]
        bshard = c // 4
        t0 = 512 * (c % 4)
        in_maps.append(
            {
                "x_all": resid,
                "resid_mine": np.ascontiguousarray(resid[bshard, t0 : t0 + TSH]),
                "wqkv": wqkv_host,
                "bqkv": bqkv_host,
                "wo": wo_host,
                "win": win_host,
                "bin": bin_host,
                "wout": wout_host,
                "bout": bout_host,
                "masks": masks,
                "ident": ident,
                "ones": ones_host,
                "vinit": vinit_host,
            }
        )
    return in_maps


_NC = None


def _get_nc():
    global _NC
    if _NC is None:
        _NC = build_nc()
    return _NC


def run_on_cores(in_maps):
    nc = _get_nc()
    return run_bass_kernel_spmd(nc, in_maps, core_ids=list(range(N_CORES)))


def kernel(**inputs) -> np.ndarray:
    in_maps = _prep_inputs(inputs)
    res = run_on_cores(in_maps)
    out = np.zeros((B, S, D), np.float32)
    for c in range(N_CORES):
        out[c // 4, 512 * (c % 4) : 512 * (c % 4) + TSH] = res.results[c]["y"]
    return out


if __name__ == "__main__":
    # quick self-exercise with random data
    rng = np.random.default_rng(0)
    ins = {
        "resid": rng.standard_normal((B, S, D)).astype(np.float32),
        "w_q": 0.02 * rng.standard_normal((H, D, DH)).astype(np.float32),
        "w_k": 0.02 * rng.standard_normal((H, D, DH)).astype(np.float32),
        "w_v": 0.02 * rng.standard_normal((H, D, DH)).astype(np.float32),
        "w_o": 0.02 * rng.standard_normal((H, DH, D)).astype(np.float32),
        "ln1_w": 0.02 * rng.standard_normal(D).astype(np.float32),
        "ln1_b": np.zeros(D, np.float32),
        "ln2_w": 0.02 * rng.standard_normal(D).astype(np.float32),
        "ln2_b": np.zeros(D, np.float32),
        "w_in": 0.02 * rng.standard_normal((D, DM)).astype(np.float32),
        "b_in": np.zeros(DM, np.float32),
        "w_out": 0.02 * rng.standard_normal((DM, D)).astype(np.float32),
        "b_out": np.zeros(D, np.float32),
    }
    out = kernel(**ins)
    print("out", out.shape, out.dtype, float(np.abs(out).mean()))


# revision 33
# speedup vs baseline: 38.3611x; 38.3611x over previous
"""Trainium2 Bass kernel for a dense transformer block (nn_Block_29583734734992).

Reference computation (fp32):
    resid = resid + Attn(LN1(resid))          # 16 heads, d_head 64, causal
    resid = resid + MLP(LN2(resid)) + b_out   # d_mlp 4096, tanh-gelu

Sharding over 8 NeuronCores:
  - Attention phase: head-parallel. Core c computes heads (2c, 2c+1) for BOTH
    batch elements over the full 2048-token sequence. Scores/softmax/z are
    computed per head in transposed layout (d on partitions).
  - One 8-rank AllToAll reshards z from head-major to token-major: token shard
    j = (batch j//4, tokens 512*(j%4) .. +512). Core c ends up with all 16
    heads for its 512-token shard.
  - Post phase: token-parallel. Each core does the o-projection, residual add,
    LN2 and the full MLP for its 512 tokens, writing a [512, 1024] output
    shard that the host reassembles.

Numerics: all matmuls run as float32r (1 cycle/row for free dim >= 256);
LN scale/bias, the 1/sqrt(64) softmax scale and b_in are folded into weights /
activation biases on the host. Softmax skips max-subtraction (scores are tiny)
and applies the causal mask multiplicatively after exp; the per-query softmax
denominator comes from an extra ones-column appended to V's stationary operand.
"""

import sys

for _p in ("/opt/trn_rl_repo", "/root/.axon_site/_ro/trn_rl_repo"):
    if _p not in sys.path:
        sys.path.insert(0, _p)

import ml_dtypes
import numpy as np

import concourse.bass as bass
import concourse.mybir as mybir
import concourse.tile as tile
from concourse import bacc
from concourse.bass_utils import run_bass_kernel_spmd

F32 = mybir.dt.float32
F32R = mybir.dt.float32r
BF16 = mybir.dt.bfloat16
AF = mybir.ActivationFunctionType
OP = mybir.AluOpType

N_CORES = 8
B, S, D = 2, 2048, 1024
H, DH, DM = 16, 64, 4096
EPS = 1e-5
HPC = H // N_CORES  # heads per core = 2
TSH = (B * S) // N_CORES  # tokens per core post-A2A = 512
NT = S // 128  # 16 token tiles per batch
ND = D // 128  # 8 d_model chunks
NM = DM // 128  # 32 d_mlp chunks
NQC = S // 512  # 4 query chunks of 512
NKC = S // 128  # 16 key chunks of 128

# Set True to compute gelu explicitly as x + x*tanh-part (0.5 folded into
# w_out) instead of the ACT Gelu_apprx_tanh LUT.
EXPLICIT_GELU = False


def build_nc():
    nc = bacc.Bacc("TRN2", target_bir_lowering=False, debug=False, num_devices=N_CORES)

    x_all = nc.dram_tensor("x_all", [B, S, D], F32, kind="ExternalInput")
    resid_mine = nc.dram_tensor("resid_mine", [TSH, D], F32, kind="ExternalInput")
    wqkv = nc.dram_tensor("wqkv", [ND, 128, 3 * 2 * DH], F32R, kind="ExternalInput")
    bqkv = nc.dram_tensor("bqkv", [128, 3], F32, kind="ExternalInput")
    wo = nc.dram_tensor("wo", [ND, 128, D], F32R, kind="ExternalInput")
    win = nc.dram_tensor("win", [NM, ND, 128, 128], F32R, kind="ExternalInput")
    bin_ = nc.dram_tensor("bin", [NM, 128, 1], F32, kind="ExternalInput")
    wout = nc.dram_tensor("wout", [NM, 128, D], BF16, kind="ExternalInput")
    bout = nc.dram_tensor("bout", [1, D], F32R, kind="ExternalInput")
    masks = nc.dram_tensor("masks", [4, 128, 512], F32R, kind="ExternalInput")
    ident = nc.dram_tensor("ident", [128, 128], F32, kind="ExternalInput")
    ones = nc.dram_tensor("ones", [1, 128], F32R, kind="ExternalInput")
    vinit = nc.dram_tensor("vinit", [128, HPC, NKC, DH + 1], F32R, kind="ExternalInput")
    y = nc.dram_tensor("y", [TSH, D], F32, kind="ExternalOutput")

    with tile.TileContext(nc) as tc:
        with (
            tc.tile_pool(name="singles", bufs=1) as singles,
            tc.tile_pool(name="dram", bufs=1, space="DRAM") as dram,
        ):
            a2a_in = dram.tile([N_CORES, HPC * DH, 512], F32R)
            a2a_out = dram.tile([N_CORES, HPC * DH, 512], F32R)

            ident_sb = singles.tile([128, 128], F32)
            nc.sync.dma_start(ident_sb[:], ident[:])
            mask_sb = singles.tile([128, 4, 512], F32R)
            for p in range(4):
                nc.sync.dma_start(mask_sb[:, p, :], masks[p])
            wqkv_sb = singles.tile([128, ND, 3 * 2 * DH], F32R)
            nc.sync.dma_start(wqkv_sb[:], wqkv.rearrange("c p f -> p c f"))
            bqkv_sb = singles.tile([128, 3], F32)
            nc.sync.dma_start(bqkv_sb[:], bqkv[:])
            eps_sb = singles.tile([128, 1], F32)
            nc.vector.memset(eps_sb[:], EPS)
            bout_sb = singles.tile([1, D], F32R)
            nc.sync.dma_start(bout_sb[:], bout[:])
            ones_sb = singles.tile([1, 128], F32R)
            nc.sync.dma_start(ones_sb[:], ones[:])

            # ---------------- attention phase (head-parallel) ----------------
            with (
                tc.tile_pool(name="attn_x", bufs=2) as axp,
                tc.tile_pool(name="attn_big", bufs=1) as abig,
                tc.tile_pool(name="attn_qkv", bufs=2) as aqkv,
                tc.tile_pool(name="attn_vT", bufs=1) as avt,
                tc.tile_pool(name="attn_sm", bufs=3) as asm,
                tc.tile_pool(name="attn_ps", bufs=2, space="PSUM") as aps,
            ):
                for b in range(B):
                    xlnT = abig.tile([128, ND, S], F32R, tag="xlnT")
                    # LN1 over all tokens of batch b, then transpose into xlnT
                    for tci in range(NT):
                        xt = axp.tile([128, D], F32, tag="xt")
                        nc.sync.dma_start(xt[:], x_all[b, tci * 128 : (tci + 1) * 128, :])
                        stats = axp.tile([128, 2, 6], F32, tag="stats")
                        nc.vector.bn_stats(stats[:, 0, :], xt[:, 0:512])
                        nc.vector.bn_stats(stats[:, 1, :], xt[:, 512:1024])
                        mv = axp.tile([128, 2], F32, tag="mv")
                        nc.vector.bn_aggr(mv[:], stats[:])
                        std = axp.tile([128, 1], F32, tag="std")
                        nc.scalar.activation(std[:], mv[:, 1:2], AF.Sqrt, bias=eps_sb[:])
                        rstd = axp.tile([128, 1], F32, tag="rstd")
                        nc.vector.reciprocal(rstd[:], std[:])
                        xln = axp.tile([128, D], F32, tag="xln")
                        nc.vector.tensor_scalar(
                            out=xln[:],
                            in0=xt[:],
                            scalar1=mv[:, 0:1],
                            scalar2=rstd[:],
                            op0=OP.subtract,
                            op1=OP.mult,
                        )
                        for dc in range(ND):
                            tp = aps.tile([128, 128], F32, tag="smallps")
                            nc.tensor.transpose(
                                tp[:], xln[:, dc * 128 : (dc + 1) * 128], ident_sb[:]
                            )
                            nc.vector.tensor_copy(
                                xlnT[:, dc, tci * 128 : (tci + 1) * 128], tp[:]
                            )

                    # QKV^T for this core's 2 heads: [128 (2h x 64), S] each
                    qT = aqkv.tile([128, S], F32R, tag="qT")
                    kT = aqkv.tile([128, S], F32R, tag="kT")
                    vT = avt.tile([128, S], F32, tag="vT")
                    dests = [qT, kT, vT]
                    for p in range(3):
                        for tcol in range(NQC):
                            ps = aps.tile([128, 512], F32, tag="qkvps")
                            for dc in range(ND):
                                nc.tensor.matmul(
                                    ps[:],
                                    wqkv_sb[:, dc, p * 128 : (p + 1) * 128],
                                    xlnT[:, dc, tcol * 512 : (tcol + 1) * 512],
                                    start=(dc == 0),
                                    stop=(dc == ND - 1),
                                )
                            nc.vector.tensor_scalar_add(
                                out=dests[p][:, tcol * 512 : (tcol + 1) * 512],
                                in0=ps[:],
                                scalar1=bqkv_sb[:, p : p + 1],
                            )

                    # V token-major with an appended ones column: [128k, kc, 65]
                    vt = aqkv.tile([128, HPC, NKC, DH + 1], F32R, tag="vt")
                    nc.sync.dma_start(vt[:], vinit[:])
                    for h in range(HPC):
                        for kc in range(NKC):
                            tp = aps.tile([128, 128], F32, tag="smallps")
                            nc.tensor.transpose(
                                tp[:, 0:DH],
                                vT[h * DH : (h + 1) * DH, kc * 128 : (kc + 1) * 128],
                                ident_sb[h * DH : (h + 1) * DH, h * DH : (h + 1) * DH],
                            )
                            nc.vector.tensor_copy(vt[:, h, kc, 0:DH], tp[:, 0:DH])

                    # causal attention, z^T per head, normalize, stage for A2A
                    for h in range(HPC):
                        hs = slice(h * DH, (h + 1) * DH)
                        for qc in range(NQC):
                            zp = aps.tile([DH + 1, 512], F32, tag="zpsum")
                            nkc = 4 * qc + 4
                            for kc in range(nkc):
                                sp = aps.tile([128, 512], F32, tag="spsum")
                                nc.tensor.matmul(
                                    sp[:],
                                    kT[hs, kc * 128 : (kc + 1) * 128],
                                    qT[hs, qc * 512 : (qc + 1) * 512],
                                    start=True,
                                    stop=True,
                                )
                                es = asm.tile([128, 512], F32R, tag="es")
                                nc.scalar.activation(es[:], sp[:], AF.Exp)
                                if kc >= 4 * qc:
                                    nc.gpsimd.tensor_tensor(
                                        es[:], es[:], mask_sb[:, kc - 4 * qc, :], OP.mult
                                    )
                                nc.tensor.matmul(
                                    zp[:],
                                    vt[:, h, kc, :],
                                    es[:],
                                    start=(kc == 0),
                                    stop=(kc == nkc - 1),
                                )
                            recip = asm.tile([1, 512], F32R, tag="recip")
                            with nc.allow_low_precision(
                                reason="fp32r softmax denom; rounding loss is ~1e-7 rel"
                            ):
                                nc.vector.reciprocal(recip[:], zp[DH : DH + 1, :])
                            bc = aps.tile([DH, 512], F32, tag="smallps")
                            nc.tensor.matmul(
                                bc[:],
                                ones_sb[:, 0:DH],
                                recip[:],
                                start=True,
                                stop=True,
                            )
                            zn = asm.tile([DH, 512], F32R, tag="zn")
                            with nc.allow_low_precision(
                                reason="fp32r z output; rounding loss is ~1e-7 rel"
                            ):
                                nc.vector.tensor_copy(zn[:], zp[0:DH, :])
                                nc.vector.tensor_tensor(zn[:], zn[:], bc[:], OP.mult)
                            nc.sync.dma_start(
                                a2a_in[b * 4 + qc, h * DH : (h + 1) * DH, :], zn[:]
                            )

            nc.gpsimd.collective_compute(
                "AllToAll",
                OP.bypass,
                replica_groups=[list(range(N_CORES))],
                ins=[a2a_in[:]],
                outs=[a2a_out[:]],
            )

            # ---------------- post phase (token-parallel) ----------------
            with (
                tc.tile_pool(name="post_w", bufs=3) as pw,
                tc.tile_pool(name="post_big", bufs=1) as pbig,
                tc.tile_pool(name="post_t", bufs=3) as pt,
                tc.tile_pool(name="post_ps", bufs=2, space="PSUM") as pps,
            ):
                resid2 = pbig.tile([128, 4, D], F32, tag="resid2")

                # broadcast b_out across partitions once: [1, D] -> [128, D]
                bout_full = pbig.tile([128, D], F32, tag="bout_full")
                for dc2 in range(2):
                    bps = pps.tile([128, 512], F32, tag="ps1")
                    nc.tensor.matmul(
                        bps[:],
                        ones_sb[:],
                        bout_sb[:, dc2 * 512 : (dc2 + 1) * 512],
                        start=True,
                        stop=True,
                    )
                    nc.vector.tensor_copy(bout_full[:, dc2 * 512 : (dc2 + 1) * 512], bps[:])

                # o-projection + residual add -> resid2
                with tc.tile_pool(name="post_o", bufs=1) as po:
                    zt = po.tile([128, N_CORES, 512], F32R, tag="zt")
                    for i in range(N_CORES):
                        nc.sync.dma_start(zt[:, i, :], a2a_out[i])
                    wo_sb = po.tile([128, ND, D], F32R, tag="wo")
                    nc.sync.dma_start(wo_sb[:], wo.rearrange("c p f -> p c f"))
                    for tsub in range(4):
                        for dc2 in range(2):
                            op_ = pps.tile([128, 512], F32, tag="ps1")
                            for hd in range(ND):
                                nc.tensor.matmul(
                                    op_[:],
                                    zt[:, hd, tsub * 128 : (tsub + 1) * 128],
                                    wo_sb[:, hd, dc2 * 512 : (dc2 + 1) * 512],
                                    start=(hd == 0),
                                    stop=(hd == ND - 1),
                                )
                            rs = pt.tile([128, 512], F32, tag="rs")
                            nc.sync.dma_start(
                                rs[:],
                                resid_mine[
                                    tsub * 128 : (tsub + 1) * 128,
                                    dc2 * 512 : (dc2 + 1) * 512,
                                ],
                            )
                            nc.vector.tensor_tensor(
                                resid2[:, tsub, dc2 * 512 : (dc2 + 1) * 512],
                                op_[:],
                                rs[:],
                                OP.add,
                            )

                # LN2 + transpose -> xln2T [128, ND, 512]
                xln2T = pbig.tile([128, ND, 512], F32R, tag="xln2T")
                for tsub in range(4):
                    stats = pt.tile([128, 2, 6], F32, tag="stats2")
                    nc.vector.bn_stats(stats[:, 0, :], resid2[:, tsub, 0:512])
                    nc.vector.bn_stats(stats[:, 1, :], resid2[:, tsub, 512:1024])
                    mv = pt.tile([128, 2], F32, tag="mv2")
                    nc.vector.bn_aggr(mv[:], stats[:])
                    std = pt.tile([128, 1], F32, tag="std2")
                    nc.scalar.activation(std[:], mv[:, 1:2], AF.Sqrt, bias=eps_sb[:])
                    rstd = pt.tile([128, 1], F32, tag="rstd2")
                    nc.vector.reciprocal(rstd[:], std[:])
                    xln2 = pt.tile([128, D], F32, tag="xln2")
                    nc.vector.tensor_scalar(
                        out=xln2[:],
                        in0=resid2[:, tsub, :],
                        scalar1=mv[:, 0:1],
                        scalar2=rstd[:],
                        op0=OP.subtract,
                        op1=OP.mult,
                    )
                    for dc in range(ND):
                        tp = pps.tile([128, 128], F32, tag="ps1")
                        nc.tensor.transpose(
                            tp[:], xln2[:, dc * 128 : (dc + 1) * 128], ident_sb[:]
                        )
                        nc.vector.tensor_copy(
                            xln2T[:, dc, tsub * 128 : (tsub + 1) * 128], tp[:]
                        )

                # MLP pass A: h1^T per m-chunk -> gelu -> gT; accumulate out d 0:512
                gT = pbig.tile([128, NM, 512], BF16, tag="gT")
                acc = pps.tile([128, 4, 512], F32, tag="acc", bufs=1)
                for m in range(NM):
                    wi = pw.tile([128, ND, 128], F32R, tag="wi")
                    nc.sync.dma_start(wi[:], win[m].rearrange("c p f -> p c f"))
                    bi = pw.tile([128, 1], F32, tag="bi")
                    nc.sync.dma_start(bi[:], bin_[m])
                    h1 = pps.tile([128, 512], F32, tag="ps1")
                    for dc in range(ND):
                        nc.tensor.matmul(
                            h1[:],
                            wi[:, dc, :],
                            xln2T[:, dc, :],
                            start=(dc == 0),
                            stop=(dc == ND - 1),
                        )
                    if EXPLICIT_GELU:
                        # g = x*(1 + tanh(c*(x + a*x^3))), with x = h1 + b_in
                        # (the 0.5 is folded into w_out on the host)
                        xg = pt.tile([128, 512], F32, tag="xg")
                        nc.vector.tensor_scalar_add(out=xg[:], in0=h1[:], scalar1=bi[:])
                        x2 = pt.tile([128, 512], F32, tag="x2")
                        nc.vector.tensor_tensor(x2[:], xg[:], xg[:], OP.mult)
                        nc.vector.tensor_scalar(
                            out=x2[:],
                            in0=x2[:],
                            scalar1=0.044715,
                            scalar2=1.0,
                            op0=OP.mult,
                            op1=OP.add,
                        )
                        nc.vector.tensor_tensor(x2[:], x2[:], xg[:], OP.mult)
                        th = pt.tile([128, 512], F32, tag="th")
                        nc.scalar.activation(
                            th[:], x2[:], AF.Tanh, scale=0.7978845608028654
                        )
                        nc.vector.tensor_scalar_add(out=th[:], in0=th[:], scalar1=1.0)
                        nc.vector.tensor_tensor(gT[:, m, :], th[:], xg[:], OP.mult)
                    else:
                        nc.scalar.activation(
                            gT[:, m, :], h1[:], AF.Gelu_apprx_tanh, bias=bi[:]
                        )
                    wu = pw.tile([128, 512], BF16, tag="wu")
                    nc.sync.dma_start(wu[:], wout[m, :, 0:512])
                    for tsub in range(4):
                        nc.tensor.matmul(
                            acc[:, tsub, :],
                            gT[:, m, tsub * 128 : (tsub + 1) * 128],
                            wu[:],
                            start=(m == 0),
                            stop=(m == NM - 1),
                        )
                for tsub in range(4):
                    ot = pt.tile([128, 512], F32, tag="ot")
                    nc.vector.tensor_tensor(
                        ot[:], acc[:, tsub, :], resid2[:, tsub, 0:512], OP.add
                    )
                    nc.vector.tensor_tensor(ot[:], ot[:], bout_full[:, 0:512], OP.add)
                    nc.sync.dma_start(
                        y[tsub * 128 : (tsub + 1) * 128, 0:512], ot[:]
                    )

                # MLP pass B: same gT, out d 512:1024
                acc2 = pps.tile([128, 4, 512], F32, tag="acc", bufs=1)
                for m in range(NM):
                    wu = pw.tile([128, 512], BF16, tag="wu")
                    nc.sync.dma_start(wu[:], wout[m, :, 512:1024])
                    for tsub in range(4):
                        nc.tensor.matmul(
                            acc2[:, tsub, :],
                            gT[:, m, tsub * 128 : (tsub + 1) * 128],
                            wu[:],
                            start=(m == 0),
                            stop=(m == NM - 1),
                        )
                for tsub in range(4):
                    ot = pt.tile([128, 512], F32, tag="ot")
                    nc.vector.tensor_tensor(
                        ot[:], acc2[:, tsub, :], resid2[:, tsub, 512:1024], OP.add
                    )
                    nc.vector.tensor_tensor(ot[:], ot[:], bout_full[:, 512:1024], OP.add)
                    nc.sync.dma_start(
                        y[tsub * 128 : (tsub + 1) * 128, 512:1024], ot[:]
                    )

    nc.compile()
    return nc


def _prep_inputs(inputs):
    """Host-side weight folding; returns per-core in_maps."""
    f32 = np.float32
    resid = np.asarray(inputs["resid"], f32)
    w_q = np.asarray(inputs["w_q"], f32)
    w_k = np.asarray(inputs["w_k"], f32)
    w_v = np.asarray(inputs["w_v"], f32)
    w_o = np.asarray(inputs["w_o"], f32)
    ln1_w = np.asarray(inputs["ln1_w"], f32)
    ln1_b = np.asarray(inputs["ln1_b"], f32)
    ln2_w = np.asarray(inputs["ln2_w"], f32)
    ln2_b = np.asarray(inputs["ln2_b"], f32)
    w_in = np.asarray(inputs["w_in"], f32)
    b_in = np.asarray(inputs["b_in"], f32)
    w_out = np.asarray(inputs["w_out"], f32)
    b_out = np.asarray(inputs["b_out"], f32)

    sm = 1.0 / np.sqrt(DH)
    win_f = ln2_w[:, None] * w_in  # [D, DM]
    bin_f = ln2_b @ w_in + b_in  # [DM]
    wout_f = (0.5 * w_out if EXPLICIT_GELU else w_out).astype(f32)  # [DM, D]

    win_host = np.ascontiguousarray(
        win_f.reshape(ND, 128, NM, 128).transpose(2, 0, 1, 3)
    )
    bin_host = np.ascontiguousarray(bin_f.reshape(NM, 128, 1))
    wout_host = np.ascontiguousarray(
        wout_f.reshape(NM, 128, D).astype(ml_dtypes.bfloat16)
    )
    wo_host = np.ascontiguousarray(
        w_o.reshape(H * DH, D).reshape(ND, 128, D)
    )
    bout_host = np.ascontiguousarray(b_out.reshape(1, D))

    ones_host = np.ones((1, 128), f32)
    vinit_host = np.zeros((128, HPC, NKC, DH + 1), f32)
    vinit_host[:, :, :, DH] = 1.0

    masks = np.zeros((4, 128, 512), f32)
    for p in range(4):
        kk = 128 * p + np.arange(128)[:, None]
        qq = np.arange(512)[None, :]
        masks[p] = (kk <= qq).astype(f32)
    ident = np.eye(128, dtype=f32)

    in_maps = []
    for c in range(N_CORES):
        h0 = HPC * c
        wq2 = np.concatenate([w_q[h0 + i] for i in range(HPC)], axis=1)  # [D, 128]
        wk2 = np.concatenate([w_k[h0 + i] for i in range(HPC)], axis=1)
        wv2 = np.concatenate([w_v[h0 + i] for i in range(HPC)], axis=1)
        wq_f = ln1_w[:, None] * wq2 * sm
        wk_f = ln1_w[:, None] * wk2
        wv_f = ln1_w[:, None] * wv2
        bq = (ln1_b @ wq2) * sm
        bk = ln1_b @ wk2
        bv = ln1_b @ wv2
        wqkv_host = np.ascontiguousarray(
            np.concatenate([wq_f, wk_f, wv_f], axis=1).reshape(ND, 128, 384)
        )
        bqkv_host = np.ascontiguousarray(np.stack([bq, bk, bv], axis=1))  # [128, 3]
        bshard = c // 4
        t0 = 512 * (c % 4)
        in_maps.append(
            {
                "x_all": resid,
                "resid_mine": np.ascontiguousarray(resid[bshard, t0 : t0 + TSH]),
                "wqkv": wqkv_host,
                "bqkv": bqkv_host,
                "wo": wo_host,
                "win": win_host,
                "bin": bin_host,
                "wout": wout_host,
                "bout": bout_host,
                "masks": masks,
                "ident": ident,
                "ones": ones_host,
                "vinit": vinit_host,
            }
        )
    return in_maps


class _Runner:
    """Compile once; keep the jitted shard_map callable and device-resident
    inputs so repeat executes measure the kernel, not host overhead."""

    def __init__(self):
        import jax
        from concourse import bass2jax

        self.jax = jax
        self.bass2jax = bass2jax
        bass2jax.install_neuronx_cc_hook()
        nc = build_nc()
        self.nc = nc

        in_names, out_names, out_avals, zero_shapes = [], [], [], []
        for alloc in nc.m.functions[0].allocations:
            if not isinstance(alloc, mybir.MemoryLocationSet):
                continue
            name = alloc.memorylocations[0].name
            if alloc.kind == "ExternalInput":
                if not (nc.partition_id_tensor and name == nc.partition_id_tensor.name):
                    in_names.append(name)
            elif alloc.kind == "ExternalOutput":
                shape = tuple(alloc.tensor_shape)
                dtype = mybir.dt.np(alloc.dtype)
                out_names.append(name)
                out_avals.append(jax.core.ShapedArray(shape, dtype))
                zero_shapes.append((shape, dtype))
        n_params = len(in_names)
        all_in_names = list(in_names) + list(out_names)
        partition_name = (
            nc.partition_id_tensor.name if nc.partition_id_tensor else None
        )
        if partition_name is not None:
            all_in_names.append(partition_name)
        self.in_names = in_names
        self.out_names = out_names
        self.zero_shapes = zero_shapes
        n_outs = len(out_names)

        def _body(*args):
            operands = list(args)
            if partition_name is not None:
                operands.append(bass2jax.partition_id_tensor())
            outs = bass2jax._bass_exec_p.bind(
                *operands,
                out_avals=tuple(out_avals),
                in_names=tuple(all_in_names),
                out_names=tuple(out_names),
                lowering_input_output_aliases=(),
                sim_require_finite=True,
                sim_require_nnan=True,
                nc=nc,
            )
            return tuple(outs)

        from jax.sharding import Mesh, NamedSharding, PartitionSpec
        from jax.experimental.shard_map import shard_map

        devices = jax.devices()[:N_CORES]
        self.mesh = Mesh(np.asarray(devices), ("core",))
        self.sharding = NamedSharding(self.mesh, PartitionSpec("core"))
        donate = tuple(range(n_params, n_params + n_outs))
        in_specs = (PartitionSpec("core"),) * (n_params + n_outs)
        out_specs = (PartitionSpec("core"),) * n_outs
        self.sharded = jax.jit(
            shard_map(
                _body,
                mesh=self.mesh,
                in_specs=in_specs,
                out_specs=out_specs,
                check_rep=False,
            ),
            donate_argnums=donate,
            keep_unused=True,
        )

    def put_inputs(self, in_maps):
        concat = [
            np.concatenate([np.asarray(m[name]) for m in in_maps], axis=0)
            for name in self.in_names
        ]
        return [self.jax.device_put(a, self.sharding) for a in concat]

    def _zeros(self):
        return [
            np.zeros((N_CORES * s[0], *s[1:]), dt) for (s, dt) in self.zero_shapes
        ]

    def execute(self, dev_in):
        outs = self.sharded(*dev_in, *self._zeros())
        for o in outs:
            o.block_until_ready()
        return outs

    def gather(self, outs):
        per_core = {}
        for i, name in enumerate(self.out_names):
            arr = np.asarray(outs[i])
            per_core[name] = arr.reshape(N_CORES, -1, *arr.shape[1:])
        return per_core


_RUNNER = None


def _get_runner():
    global _RUNNER
    if _RUNNER is None:
        _RUNNER = _Runner()
    return _RUNNER


def kernel(**inputs) -> np.ndarray:
    r = _get_runner()
    in_maps = _prep_inputs(inputs)
    dev_in = r.put_inputs(in_maps)
    outs = r.execute(dev_in)
    ys = r.gather(outs)["y"]  # [8, 512, 1024]
    out = np.zeros((B, S, D), np.float32)
    for c in range(N_CORES):
        out[c // 4, 512 * (c % 4) : 512 * (c % 4) + TSH] = ys[c]
    return out


if __name__ == "__main__":
    # quick self-exercise with random data
    rng = np.random.default_rng(0)
    ins = {
        "resid": rng.standard_normal((B, S, D)).astype(np.float32),
        "w_q": 0.02 * rng.standard_normal((H, D, DH)).astype(np.float32),
        "w_k": 0.02 * rng.standard_normal((H, D, DH)).astype(np.float32),
        "w_v": 0.02 * rng.standard_normal((H, D, DH)).astype(np.float32),
        "w_o": 0.02 * rng.standard_normal((H, DH, D)).astype(np.float32),
        "ln1_w": 0.02 * rng.standard_normal(D).astype(np.float32),
        "ln1_b": np.zeros(D, np.float32),
        "ln2_w": 0.02 * rng.standard_normal(D).astype(np.float32),
        "ln2_b": np.zeros(D, np.float32),
        "w_in": 0.02 * rng.standard_normal((D, DM)).astype(np.float32),
        "b_in": np.zeros(DM, np.float32),
        "w_out": 0.02 * rng.standard_normal((DM, D)).astype(np.float32),
        "b_out": np.zeros(D, np.float32),
    }
    out = kernel(**ins)
    print("out", out.shape, out.dtype, float(np.abs(out).mean()))
